# revision 21
# baseline (speedup 1.0000x reference)
"""Trainium2 Bass kernel: elementary cellular automaton (Wolfram rule table).

Problem: state (16, 1, 32768) of 0/1 floats, 8-entry rule table, evolve
`steps` times, return the full trajectory (16, steps+1, 1, 32768) fp32.

Sharding: data-parallel over batch B=16 across 8 NeuronCores (2 rows/core).

Per-core layout (partition-major): row cell w = p*256 + j lives at SBUF
partition p, free-dim col G+j of a width-(256+2G) block. The 2G ghost
columns replicate the tails of the neighboring partitions, so G steps run
back-to-back on the DVE with shrinking valid windows and no cross-partition
traffic; every G steps the ghosts are rebuilt with two small PE permutation
matmuls (partition rotate via a 128x128 one-hot matrix) + one ACT copy.

Rule-110 step = 2 DVE ops on both batch rows fused into one AP:
    t  = max(l, 0.5) * c     (scalar_tensor_tensor: (l max 0.5) mult c)
    ns = (t != r)            (tensor_tensor not_equal)
(t encodes c=0 -> 0 ; c=1,l=0 -> 0.5 ; c=1,l=1 -> 1, so ns matches the
rule-110 table for binary states.) Generic binary tables fall back to
idx = 4l+2c+r then a sum of is_equal matches against the table's set bits.

Output staging: each step's new state lands in a block of a staging tile;
one ~1 MiB DMA per row flushes KF steps at a time.
"""

import os
import sys
import types
import numpy as np

P = 128            # SBUF partitions
N_CORES = 8
B_FULL = 16
ROWS = B_FULL // N_CORES   # batch rows per core
W = 32768
RULE110 = (0, 1, 1, 1, 0, 1, 1, 0)

_PROGRAM_CACHE = {}
_LAST_RESULTS = None   # BassKernelResults of the most recent run (for tests)


def _install_ntff_shim():
    """Register the axon NTFF profiling hook if the image's antenv lacks it."""
    try:
        from antenv.axon_hooks import get_axon_ntff_profile_hook  # noqa: F401
        return
    except ImportError:
        pass
    try:
        import antenv
        import trn_agent_boot.trn_boot as tb
        mod = types.ModuleType("antenv.axon_hooks")
        _hook = [None]
        mod.set_axon_ntff_profile_hook = lambda h: _hook.__setitem__(0, h)
        mod.get_axon_ntff_profile_hook = lambda: _hook[0]
        sys.modules["antenv.axon_hooks"] = mod
        antenv.axon_hooks = mod
        mod.set_axon_ntff_profile_hook(
            tb._ntff_profile_via_ctypes("/opt/axon/libaxon_pjrt.so")
        )
    except Exception:
        pass


def _build_program(steps, table_bits, w=W, ghost=16, kf=8, gp_frac=0.0):
    import concourse.bacc as bacc
    from concourse import mybir, tile
    from concourse.alu_op_type import AluOpType as aop

    f32 = mybir.dt.float32
    C = w // P                 # cells per partition per row
    G = ghost
    WB = C + 2 * G             # block width per row (ghosts on both sides)
    BLK = ROWS * WB            # one step-block, both rows

    nc = bacc.Bacc("TRN2", target_bir_lowering=False, debug=False,
                   num_devices=N_CORES)
    x = nc.dram_tensor("x", [ROWS, w], f32, kind="ExternalInput").ap()
    pl = nc.dram_tensor("pl", [P, P], f32, kind="ExternalInput").ap()
    pr = nc.dram_tensor("pr", [P, P], f32, kind="ExternalInput").ap()
    y = nc.dram_tensor("y", [ROWS, steps + 1, w], f32, kind="ExternalOutput").ap()

    rule110 = tuple(table_bits) == RULE110
    set_bits = [i for i, b in enumerate(table_bits) if b]

    with tile.TileContext(nc) as tc:
        with (
            tc.tile_pool(name="const", bufs=1) as const_pool,
            tc.tile_pool(name="stage", bufs=3) as stage_pool,
            tc.tile_pool(name="tmp", bufs=2) as tmp_pool,
            tc.tile_pool(name="ps", bufs=2, space="PSUM") as ps_pool,
        ):
            plt = const_pool.tile([P, P], f32, tag="pl")
            prt = const_pool.tile([P, P], f32, tag="pr")
            nc.sync.dma_start(out=plt[:, :], in_=pl)
            nc.sync.dma_start(out=prt[:, :], in_=pr)


            def refresh_ghosts(blk4):
                # blk4: (P, ROWS, WB) with valid interior [G, G+C).
                # left ghosts  [0,G):    blk4[p, r, q]      = blk4[p-1, r, C+q]
                # right ghosts [WB-G,WB): blk4[p, r, WB-G+q] = blk4[p+1, r, G+q]
                ps = ps_pool.tile([P, 2 * ROWS * G], f32, tag="ps")
                nc.tensor.matmul(ps[:, 0:ROWS * G], plt[:, :],
                                 blk4[:, :, C:C + G], start=True, stop=True)
                nc.tensor.matmul(ps[:, ROWS * G:2 * ROWS * G], prt[:, :],
                                 blk4[:, :, G:2 * G], start=True, stop=True)
                # psum col = side*(ROWS*G) + r*G + q
                src = ps[:, :].rearrange("p (s r q) -> p r s q", s=2, r=ROWS)
                nc.scalar.copy(out=blk4[:, :, 0:G],
                               in_=src[:, :, 0, :])
                nc.scalar.copy(out=blk4[:, :, WB - G:WB],
                               in_=src[:, :, 1, :])

            def emit_step(in4, out4, m, split_first=False, split_last=False):
                if rule110 and split_last:
                    # Base step of a period (m == G-1): compute the two
                    # refresh-source column ranges first so the PE/ACT ghost
                    # rebuild starts while the DVE finishes the middle.
                    assert m == G - 1
                    t = tmp_pool.tile([P, ROWS * (WB - 2)], f32, tag="t")
                    t3 = t[:, :].rearrange("p (r j) -> p r j", r=ROWS)
                    wd = WB - 2 * m - 2          # = C
                    nc.vector.scalar_tensor_tensor(
                        t3[:, :, 0:wd], in4[:, :, m:m + wd], 0.5,
                        in4[:, :, m + 1:m + 1 + wd], aop.max, aop.mult)
                    # out cols [G, 2G) then [C, C+G)   (t index = out col - G)
                    for o0 in (G, C):
                        nc.vector.tensor_tensor(
                            out4[:, :, o0:o0 + G], t3[:, :, o0 - G:o0],
                            in4[:, :, o0 + 1:o0 + G + 1], aop.not_equal)
                    return t3   # caller emits refresh, then finish_middle
                return _emit_plain(in4, out4, m, split_first)

            def finish_middle(in4, out4, t3):
                # remainder of a split_last step: out cols [2G, C)
                nc.vector.tensor_tensor(
                    out4[:, :, 2 * G:C], t3[:, :, G:C - G],
                    in4[:, :, 2 * G + 1:C + 1], aop.not_equal)

            def _emit_plain(in4, out4, m, split_first=False):
                # in4 valid [m, WB-m); out4 gets [m+1, WB-m-1)
                wd = WB - 2 * m - 2
                li = in4[:, :, m:m + wd]
                ci = in4[:, :, m + 1:m + 1 + wd]
                ri = in4[:, :, m + 2:m + 2 + wd]
                oi = out4[:, :, m + 1:m + 1 + wd]
                if rule110 and split_first:
                    # Period-boundary step (m == 0): emit the ghost-independent
                    # interior first so the DVE overlaps the PE/ACT ghost
                    # refresh, then the 2G edge columns that need the ghosts.
                    assert m == 0
                    t = tmp_pool.tile([P, ROWS * (WB - 2)], f32, tag="t")
                    t3 = t[:, :].rearrange("p (r j) -> p r j", r=ROWS)
                    # interior: out cols [G+1, G+C-1) -> t index [G, G+C-2)
                    nc.vector.scalar_tensor_tensor(
                        t3[:, :, G:G + C - 2], in4[:, :, G:G + C - 2], 0.5,
                        in4[:, :, G + 1:G + C - 1], aop.max, aop.mult)
                    nc.vector.tensor_tensor(
                        out4[:, :, G + 1:G + C - 1], t3[:, :, G:G + C - 2],
                        in4[:, :, G + 2:G + C], aop.not_equal)
                    # edges: out cols [1, G+1) and [WB-G-1, WB-1)
                    for s0 in (0, WB - G - 2):
                        nc.vector.scalar_tensor_tensor(
                            t3[:, :, s0:s0 + G], in4[:, :, s0:s0 + G], 0.5,
                            in4[:, :, s0 + 1:s0 + G + 1], aop.max, aop.mult)
                        nc.vector.tensor_tensor(
                            out4[:, :, s0 + 1:s0 + G + 1], t3[:, :, s0:s0 + G],
                            in4[:, :, s0 + 2:s0 + G + 2], aop.not_equal)
                elif rule110:
                    # Optionally split each op's column range between the DVE
                    # and GPSIMD; the two chains are fully independent.
                    a = int(wd * (1.0 - gp_frac)) if gp_frac > 0.0 else wd
                    t = tmp_pool.tile([P, ROWS * (WB - 2)], f32, tag="t")
                    t3 = t[:, :].rearrange("p (r j) -> p r j", r=ROWS)
                    nc.vector.scalar_tensor_tensor(t3[:, :, 0:a], li[:, :, 0:a],
                                                   0.5, ci[:, :, 0:a],
                                                   aop.max, aop.mult)
                    nc.vector.tensor_tensor(oi[:, :, 0:a], t3[:, :, 0:a],
                                            ri[:, :, 0:a], aop.not_equal)
                    if a < wd:
                        # Pool supports only add/mult/subtract -> use the
                        # multilinear form ns = (c+r) - cr - l*cr.
                        tg = tmp_pool.tile([P, ROWS * (WB - 2)], f32, tag="tg")
                        tg3 = tg[:, :].rearrange("p (r j) -> p r j", r=ROWS)
                        ug = tmp_pool.tile([P, ROWS * (WB - 2)], f32, tag="ug")
                        ug3 = ug[:, :].rearrange("p (r j) -> p r j", r=ROWS)
                        lg, cg, rg = (li[:, :, a:wd], ci[:, :, a:wd],
                                      ri[:, :, a:wd])
                        nc.gpsimd.tensor_tensor(tg3[:, :, a:wd], cg, rg,
                                                aop.mult)           # cr
                        nc.gpsimd.tensor_tensor(ug3[:, :, a:wd], cg, rg,
                                                aop.add)            # c+r
                        nc.gpsimd.tensor_tensor(ug3[:, :, a:wd], ug3[:, :, a:wd],
                                                tg3[:, :, a:wd],
                                                aop.subtract)       # c+r-cr
                        nc.gpsimd.tensor_tensor(tg3[:, :, a:wd], tg3[:, :, a:wd],
                                                lg, aop.mult)       # l*cr
                        nc.gpsimd.tensor_tensor(oi[:, :, a:wd], ug3[:, :, a:wd],
                                                tg3[:, :, a:wd], aop.subtract)
                elif len(set_bits) == 0:
                    nc.vector.memset(oi, 0.0)
                elif len(set_bits) == 8:
                    nc.vector.memset(oi, 1.0)
                else:
                    idx = tmp_pool.tile([P, ROWS * (WB - 2)], f32, tag="t")
                    i3 = idx[:, :].rearrange("p (r j) -> p r j", r=ROWS)[:, :, 0:wd]
                    nc.vector.scalar_tensor_tensor(i3, ci, 2.0, ri,
                                                   aop.mult, aop.add)
                    nc.vector.scalar_tensor_tensor(i3, li, 4.0, i3,
                                                   aop.mult, aop.add)
                    accs = [tmp_pool.tile([P, ROWS * (WB - 2)], f32,
                                          tag=f"acc{q}", name=f"acc{q}")
                            for q in range(2)]
                    a3 = [a[:, :].rearrange("p (r j) -> p r j", r=ROWS)[:, :, 0:wd]
                          for a in accs]
                    if len(set_bits) == 1:
                        nc.vector.tensor_scalar(oi, i3, float(set_bits[0]),
                                                None, aop.is_equal)
                    else:
                        nc.vector.tensor_scalar(a3[0], i3, float(set_bits[0]),
                                                None, aop.is_equal)
                        cur = 0
                        for kbit in set_bits[1:-1]:
                            nc.vector.scalar_tensor_tensor(
                                a3[1 - cur], i3, float(kbit), a3[cur],
                                aop.is_equal, aop.add)
                            cur = 1 - cur
                        nc.vector.scalar_tensor_tensor(
                            oi, i3, float(set_bits[-1]), a3[cur],
                            aop.is_equal, aop.add)

            # Seed block: load initial state into interior, build ghosts,
            # write y[:, 0].
            seed = const_pool.tile([P, BLK], f32, tag="seed")
            s4 = seed[:, :].rearrange("p (r j) -> p r j", r=ROWS)
            for r in range(ROWS):
                nc.sync.dma_start(out=s4[:, r, G:G + C],
                                  in_=x[r].rearrange("(p j) -> p j", p=P))
            refresh_ghosts(s4)
            for r in range(ROWS):
                yv = y[r].rearrange("t (p j) -> p t j", p=P)
                nc.sync.dma_start(out=yv[:, 0:1, :], in_=s4[:, r:r + 1, G:G + C])

            prev4 = s4
            n = 0   # states computed so far (state_n is current)
            while n < steps:
                remaining = steps - n
                if remaining > kf:
                    kk = kf
                elif remaining > 2:
                    kk = 2      # 2-step tail groups overlap the final flushes
                else:
                    kk = remaining
                st = stage_pool.tile([P, kf * BLK], f32, tag="st")
                st4 = st[:, :].rearrange("p (k r j) -> p k r j", k=kf, r=ROWS)
                for k in range(kk):
                    in4 = prev4 if k == 0 else st4[:, k - 1]
                    out4 = st4[:, k]
                    m = (n + k) % G
                    is_base = (n + k + 1) % G == 0 and n + k + 1 < steps
                    if is_base and rule110 and C >= 2 * G:
                        t3 = emit_step(in4, out4, m, split_last=True)
                        refresh_ghosts(out4)
                        finish_middle(in4, out4, t3)
                    else:
                        emit_step(in4, out4, m, split_first=(m == 0))
                        if is_base:
                            refresh_ghosts(out4)
                for r in range(ROWS):
                    yv = y[r].rearrange("t (p j) -> p t j", p=P)
                    nc.sync.dma_start(
                        out=yv[:, 1 + n:1 + n + kk, :],
                        in_=st4[:, 0:kk, r, G:G + C])
                prev4 = st4[:, kk - 1]
                n += kk

    nc.compile()
    return nc


def _perm_matrices():
    # out = lhsT.T @ rhs ; out[i] = sum_k lhsT[k, i] * rhs[k]
    pl = np.zeros((P, P), dtype=np.float32)   # out[i] = in[(i-1) % P]
    pr = np.zeros((P, P), dtype=np.float32)   # out[i] = in[(i+1) % P]
    for i in range(P):
        pl[(i - 1) % P, i] = 1.0
        pr[(i + 1) % P, i] = 1.0
    return pl, pr


def kernel(state, rule_table, steps):
    global _LAST_RESULTS
    from concourse import bass_utils

    state = np.asarray(state)
    rt = np.asarray(rule_table)
    steps = int(steps)
    bits = tuple(int(round(float(v))) for v in rt.reshape(-1))
    B = state.shape[0]
    assert B == B_FULL and state.shape[-1] == W, state.shape

    key = (steps, bits)
    if key not in _PROGRAM_CACHE:
        _PROGRAM_CACHE[key] = _build_program(steps, bits)
    nc = _PROGRAM_CACHE[key]

    x_full = np.ascontiguousarray(state.reshape(B, W).astype(np.float32))
    pl, pr = _perm_matrices()
    in_maps = [
        {"x": x_full[i * ROWS:(i + 1) * ROWS], "pl": pl, "pr": pr}
        for i in range(N_CORES)
    ]
    trace = os.environ.get("CA_TRACE") == "1"
    if trace:
        _install_ntff_shim()
    res = bass_utils.run_bass_kernel_spmd(nc, in_maps, list(range(N_CORES)),
                                          trace=trace)
    _LAST_RESULTS = res
    out = np.stack([res.results[i]["y"] for i in range(N_CORES)])
    return out.reshape(B, steps + 1, 1, W).astype(np.float32)


# revision 23
# speedup vs baseline: 1.0034x; 1.0034x over previous
"""Trainium2 Bass kernel: elementary cellular automaton (Wolfram rule table).

Problem: state (16, 1, 32768) of 0/1 floats, 8-entry rule table, evolve
`steps` times, return the full trajectory (16, steps+1, 1, 32768) fp32.

Sharding: data-parallel over batch B=16 across 8 NeuronCores (2 rows/core).

Per-core layout (partition-major): row cell w = p*256 + j lives at SBUF
partition p, free-dim col G+j of a width-(256+2G) block. The 2G ghost
columns replicate the tails of the neighboring partitions, so G steps run
back-to-back on the DVE with shrinking valid windows and no cross-partition
traffic; every G steps the ghosts are rebuilt with two small PE permutation
matmuls (partition rotate via a 128x128 one-hot matrix) + one ACT copy.

Rule-110 step = 2 DVE ops on both batch rows fused into one AP:
    t  = max(l, 0.5) * c     (scalar_tensor_tensor: (l max 0.5) mult c)
    ns = (t != r)            (tensor_tensor not_equal)
(t encodes c=0 -> 0 ; c=1,l=0 -> 0.5 ; c=1,l=1 -> 1, so ns matches the
rule-110 table for binary states.) Generic binary tables fall back to
idx = 4l+2c+r then a sum of is_equal matches against the table's set bits.

Output staging: each step's new state lands in a block of a staging tile;
one ~1 MiB DMA per row flushes KF steps at a time.
"""

import os
import sys
import types
import numpy as np

P = 128            # SBUF partitions
N_CORES = 8
B_FULL = 16
ROWS = B_FULL // N_CORES   # batch rows per core
W = 32768
RULE110 = (0, 1, 1, 1, 0, 1, 1, 0)

_PROGRAM_CACHE = {}
_LAST_RESULTS = None   # BassKernelResults of the most recent run (for tests)


def _install_ntff_shim():
    """Register the axon NTFF profiling hook if the image's antenv lacks it."""
    try:
        from antenv.axon_hooks import get_axon_ntff_profile_hook  # noqa: F401
        return
    except ImportError:
        pass
    try:
        import antenv
        import trn_agent_boot.trn_boot as tb
        mod = types.ModuleType("antenv.axon_hooks")
        _hook = [None]
        mod.set_axon_ntff_profile_hook = lambda h: _hook.__setitem__(0, h)
        mod.get_axon_ntff_profile_hook = lambda: _hook[0]
        sys.modules["antenv.axon_hooks"] = mod
        antenv.axon_hooks = mod
        mod.set_axon_ntff_profile_hook(
            tb._ntff_profile_via_ctypes("/opt/axon/libaxon_pjrt.so")
        )
    except Exception:
        pass


def _build_program(steps, table_bits, w=W, ghost=16, kf=8, gp_frac=0.0):
    import concourse.bacc as bacc
    from concourse import mybir, tile
    from concourse.alu_op_type import AluOpType as aop

    f32 = mybir.dt.float32
    C = w // P                 # cells per partition per row
    G = ghost
    WB = C + 2 * G             # block width per row (ghosts on both sides)
    BLK = ROWS * WB            # one step-block, both rows

    nc = bacc.Bacc("TRN2", target_bir_lowering=False, debug=False,
                   num_devices=N_CORES)
    x = nc.dram_tensor("x", [ROWS, w], f32, kind="ExternalInput").ap()
    pl = nc.dram_tensor("pl", [P, P], f32, kind="ExternalInput").ap()
    pr = nc.dram_tensor("pr", [P, P], f32, kind="ExternalInput").ap()
    y = nc.dram_tensor("y", [ROWS, steps + 1, w], f32, kind="ExternalOutput").ap()

    rule110 = tuple(table_bits) == RULE110
    set_bits = [i for i, b in enumerate(table_bits) if b]

    with tile.TileContext(nc) as tc:
        with (
            tc.tile_pool(name="const", bufs=1) as const_pool,
            tc.tile_pool(name="stage", bufs=3) as stage_pool,
            tc.tile_pool(name="tmp", bufs=2) as tmp_pool,
            tc.tile_pool(name="ps", bufs=2, space="PSUM") as ps_pool,
        ):
            plt = const_pool.tile([P, P], f32, tag="pl")
            prt = const_pool.tile([P, P], f32, tag="pr")
            nc.sync.dma_start(out=plt[:, :], in_=pl)
            nc.sync.dma_start(out=prt[:, :], in_=pr)


            def refresh_ghosts(blk4):
                # blk4: (P, ROWS, WB) with valid interior [G, G+C).
                # left ghosts  [0,G):    blk4[p, r, q]      = blk4[p-1, r, C+q]
                # right ghosts [WB-G,WB): blk4[p, r, WB-G+q] = blk4[p+1, r, G+q]
                ps = ps_pool.tile([P, 2 * ROWS * G], f32, tag="ps")
                nc.tensor.matmul(ps[:, 0:ROWS * G], plt[:, :],
                                 blk4[:, :, C:C + G], start=True, stop=True)
                nc.tensor.matmul(ps[:, ROWS * G:2 * ROWS * G], prt[:, :],
                                 blk4[:, :, G:2 * G], start=True, stop=True)
                # psum col = side*(ROWS*G) + r*G + q
                src = ps[:, :].rearrange("p (s r q) -> p r s q", s=2, r=ROWS)
                nc.scalar.copy(out=blk4[:, :, 0:G],
                               in_=src[:, :, 0, :])
                nc.scalar.copy(out=blk4[:, :, WB - G:WB],
                               in_=src[:, :, 1, :])

            def emit_step(in4, out4, m, split_first=False, split_last=False):
                if rule110 and split_last:
                    # Base step of a period (m == G-1): compute the two
                    # refresh-source column ranges first so the PE/ACT ghost
                    # rebuild starts while the DVE finishes the middle.
                    assert m == G - 1
                    t = tmp_pool.tile([P, ROWS * (WB - 2)], f32, tag="t")
                    t3 = t[:, :].rearrange("p (r j) -> p r j", r=ROWS)
                    wd = WB - 2 * m - 2          # = C
                    nc.vector.scalar_tensor_tensor(
                        t3[:, :, 0:wd], in4[:, :, m:m + wd], 0.5,
                        in4[:, :, m + 1:m + 1 + wd], aop.max, aop.mult)
                    # out cols [G, 2G) then [C, C+G)   (t index = out col - G)
                    for o0 in (G, C):
                        nc.vector.tensor_tensor(
                            out4[:, :, o0:o0 + G], t3[:, :, o0 - G:o0],
                            in4[:, :, o0 + 1:o0 + G + 1], aop.not_equal)
                    return t3   # caller emits refresh, then finish_middle
                return _emit_plain(in4, out4, m, split_first)

            def finish_middle(in4, out4, t3):
                # remainder of a split_last step: out cols [2G, C)
                nc.vector.tensor_tensor(
                    out4[:, :, 2 * G:C], t3[:, :, G:C - G],
                    in4[:, :, 2 * G + 1:C + 1], aop.not_equal)

            def _emit_plain(in4, out4, m, split_first=False):
                # in4 valid [m, WB-m); out4 gets [m+1, WB-m-1)
                wd = WB - 2 * m - 2
                li = in4[:, :, m:m + wd]
                ci = in4[:, :, m + 1:m + 1 + wd]
                ri = in4[:, :, m + 2:m + 2 + wd]
                oi = out4[:, :, m + 1:m + 1 + wd]
                if rule110 and split_first:
                    # Period-boundary step (m == 0): emit the ghost-independent
                    # interior first so the DVE overlaps the PE/ACT ghost
                    # refresh, then the 2G edge columns that need the ghosts.
                    assert m == 0
                    t = tmp_pool.tile([P, ROWS * (WB - 2)], f32, tag="t")
                    t3 = t[:, :].rearrange("p (r j) -> p r j", r=ROWS)
                    # interior: out cols [G+1, G+C-1) -> t index [G, G+C-2)
                    nc.vector.scalar_tensor_tensor(
                        t3[:, :, G:G + C - 2], in4[:, :, G:G + C - 2], 0.5,
                        in4[:, :, G + 1:G + C - 1], aop.max, aop.mult)
                    nc.vector.tensor_tensor(
                        out4[:, :, G + 1:G + C - 1], t3[:, :, G:G + C - 2],
                        in4[:, :, G + 2:G + C], aop.not_equal)
                    # edges: out cols [1, G+1) and [WB-G-1, WB-1)
                    for s0 in (0, WB - G - 2):
                        nc.vector.scalar_tensor_tensor(
                            t3[:, :, s0:s0 + G], in4[:, :, s0:s0 + G], 0.5,
                            in4[:, :, s0 + 1:s0 + G + 1], aop.max, aop.mult)
                        nc.vector.tensor_tensor(
                            out4[:, :, s0 + 1:s0 + G + 1], t3[:, :, s0:s0 + G],
                            in4[:, :, s0 + 2:s0 + G + 2], aop.not_equal)
                elif rule110:
                    # Optionally split each op's column range between the DVE
                    # and GPSIMD; the two chains are fully independent.
                    a = int(wd * (1.0 - gp_frac)) if gp_frac > 0.0 else wd
                    t = tmp_pool.tile([P, ROWS * (WB - 2)], f32, tag="t")
                    t3 = t[:, :].rearrange("p (r j) -> p r j", r=ROWS)
                    nc.vector.scalar_tensor_tensor(t3[:, :, 0:a], li[:, :, 0:a],
                                                   0.5, ci[:, :, 0:a],
                                                   aop.max, aop.mult)
                    nc.vector.tensor_tensor(oi[:, :, 0:a], t3[:, :, 0:a],
                                            ri[:, :, 0:a], aop.not_equal)
                    if a < wd:
                        # Pool supports only add/mult/subtract -> use the
                        # multilinear form ns = (c+r) - cr - l*cr.
                        tg = tmp_pool.tile([P, ROWS * (WB - 2)], f32, tag="tg")
                        tg3 = tg[:, :].rearrange("p (r j) -> p r j", r=ROWS)
                        ug = tmp_pool.tile([P, ROWS * (WB - 2)], f32, tag="ug")
                        ug3 = ug[:, :].rearrange("p (r j) -> p r j", r=ROWS)
                        lg, cg, rg = (li[:, :, a:wd], ci[:, :, a:wd],
                                      ri[:, :, a:wd])
                        nc.gpsimd.tensor_tensor(tg3[:, :, a:wd], cg, rg,
                                                aop.mult)           # cr
                        nc.gpsimd.tensor_tensor(ug3[:, :, a:wd], cg, rg,
                                                aop.add)            # c+r
                        nc.gpsimd.tensor_tensor(ug3[:, :, a:wd], ug3[:, :, a:wd],
                                                tg3[:, :, a:wd],
                                                aop.subtract)       # c+r-cr
                        nc.gpsimd.tensor_tensor(tg3[:, :, a:wd], tg3[:, :, a:wd],
                                                lg, aop.mult)       # l*cr
                        nc.gpsimd.tensor_tensor(oi[:, :, a:wd], ug3[:, :, a:wd],
                                                tg3[:, :, a:wd], aop.subtract)
                elif len(set_bits) == 0:
                    nc.vector.memset(oi, 0.0)
                elif len(set_bits) == 8:
                    nc.vector.memset(oi, 1.0)
                else:
                    idx = tmp_pool.tile([P, ROWS * (WB - 2)], f32, tag="t")
                    i3 = idx[:, :].rearrange("p (r j) -> p r j", r=ROWS)[:, :, 0:wd]
                    nc.vector.scalar_tensor_tensor(i3, ci, 2.0, ri,
                                                   aop.mult, aop.add)
                    nc.vector.scalar_tensor_tensor(i3, li, 4.0, i3,
                                                   aop.mult, aop.add)
                    accs = [tmp_pool.tile([P, ROWS * (WB - 2)], f32,
                                          tag=f"acc{q}", name=f"acc{q}")
                            for q in range(2)]
                    a3 = [a[:, :].rearrange("p (r j) -> p r j", r=ROWS)[:, :, 0:wd]
                          for a in accs]
                    if len(set_bits) == 1:
                        nc.vector.tensor_scalar(oi, i3, float(set_bits[0]),
                                                None, aop.is_equal)
                    else:
                        nc.vector.tensor_scalar(a3[0], i3, float(set_bits[0]),
                                                None, aop.is_equal)
                        cur = 0
                        for kbit in set_bits[1:-1]:
                            nc.vector.scalar_tensor_tensor(
                                a3[1 - cur], i3, float(kbit), a3[cur],
                                aop.is_equal, aop.add)
                            cur = 1 - cur
                        nc.vector.scalar_tensor_tensor(
                            oi, i3, float(set_bits[-1]), a3[cur],
                            aop.is_equal, aop.add)

            # Seed block: load initial state into interior, build ghosts,
            # write y[:, 0].
            seed = const_pool.tile([P, BLK], f32, tag="seed")
            s4 = seed[:, :].rearrange("p (r j) -> p r j", r=ROWS)
            for r in range(ROWS):
                nc.sync.dma_start(out=s4[:, r, G:G + C],
                                  in_=x[r].rearrange("(p j) -> p j", p=P))
            refresh_ghosts(s4)
            for r in range(ROWS):
                yv = y[r].rearrange("t (p j) -> p t j", p=P)
                nc.sync.dma_start(out=yv[:, 0:1, :], in_=s4[:, r:r + 1, G:G + C])

            prev4 = s4
            n = 0   # states computed so far (state_n is current)
            while n < steps:
                remaining = steps - n
                if remaining > kf:
                    kk = kf
                elif remaining > 2:
                    kk = 2      # 2-step tail groups overlap the final flushes
                else:
                    kk = remaining
                st = stage_pool.tile([P, kf * BLK], f32, tag="st")
                st4 = st[:, :].rearrange("p (k r j) -> p k r j", k=kf, r=ROWS)
                for k in range(kk):
                    in4 = prev4 if k == 0 else st4[:, k - 1]
                    out4 = st4[:, k]
                    m = (n + k) % G
                    is_base = (n + k + 1) % G == 0 and n + k + 1 < steps
                    if is_base and rule110 and C >= 2 * G:
                        t3 = emit_step(in4, out4, m, split_last=True)
                        refresh_ghosts(out4)
                        finish_middle(in4, out4, t3)
                    else:
                        emit_step(in4, out4, m, split_first=(m == 0))
                        if is_base:
                            refresh_ghosts(out4)
                for r in range(ROWS):
                    yv = y[r].rearrange("t (p j) -> p t j", p=P)
                    nc.sync.dma_start(
                        out=yv[:, 1 + n:1 + n + kk, :],
                        in_=st4[:, 0:kk, r, G:G + C])
                prev4 = st4[:, kk - 1]
                n += kk

    nc.compile()
    return nc


def _perm_matrices():
    # out = lhsT.T @ rhs ; out[i] = sum_k lhsT[k, i] * rhs[k]
    pl = np.zeros((P, P), dtype=np.float32)   # out[i] = in[(i-1) % P]
    pr = np.zeros((P, P), dtype=np.float32)   # out[i] = in[(i+1) % P]
    for i in range(P):
        pl[(i - 1) % P, i] = 1.0
        pr[(i + 1) % P, i] = 1.0
    return pl, pr


def kernel(state, rule_table, steps):
    global _LAST_RESULTS
    from concourse import bass_utils

    state = np.asarray(state)
    rt = np.asarray(rule_table)
    steps = int(steps)
    bits = tuple(int(round(float(v))) for v in rt.reshape(-1))
    B = state.shape[0]
    assert B == B_FULL and state.shape[-1] == W, state.shape

    key = (steps, bits)
    if key not in _PROGRAM_CACHE:
        _PROGRAM_CACHE[key] = _build_program(steps, bits)
    nc = _PROGRAM_CACHE[key]

    x_full = np.ascontiguousarray(state.reshape(B, W).astype(np.float32))
    pl, pr = _perm_matrices()
    in_maps = [
        {"x": x_full[i * ROWS:(i + 1) * ROWS], "pl": pl, "pr": pr}
        for i in range(N_CORES)
    ]
    trace = os.environ.get("CA_TRACE") == "1"
    if trace:
        _install_ntff_shim()
    res = bass_utils.run_bass_kernel_spmd(nc, in_maps, list(range(N_CORES)),
                                          trace=trace)
    _LAST_RESULTS = res
    out = np.stack([res.results[i]["y"] for i in range(N_CORES)])
    return out.reshape(B, steps + 1, 1, W).astype(np.float32)
